# revision 1
# baseline (speedup 1.0000x reference)
# Trainium2 Bass kernel for nn_MultiHeadAttentionPure (B=2, S=1024, F=1024, H=16).
#
# The reference splits q/k/v into 64 feature-chunks of 16 ("groups"), runs
# causal attention independently per (group, batch) pair -- 128 independent
# [1024,16] attention problems -- then applies a (buggy-but-faithful) torch
# reshape that scrambles (group, batch, seq) into the [B,S,F] tensor fed to
# the output linear layer.
#
# Sharding: the scramble maps output rows (b2, s2) to attention groups such
# that core c = b2*4 + q (q = s2_block of 256) needs exactly the 16 groups
# {j : j%4 == 2*b2 + q//2} at input batch b = q%2 -- a perfect partition of
# the 128 (group, batch) pairs across 8 cores with zero cross-core traffic.
# Each core computes its 16 attention groups, assembles its y^T tile
# ([1024 features, 256 rows]) on-chip, and runs the output linear for its
# 256 output rows.  Host slices inputs / concatenates outputs.
#
# On-device layout (per core, per group g):
#   scores^T[s2, s1] = k^T-slice (lhsT [16,128]) x q^T (rhs [16,512])  (fp32r)
#   causal mask: DVE adds -1e9 triangle onto PSUM before exp
#   expT = ACT Exp(PSUM) -> SBUF fp32r
#   x^T [33, s1] += vaug (lhsT [128,33], col 32 = ones) x expT   => row 32 = denom
#   recip = 1/denom (fp32r), PE-broadcast to 16 partitions via ones[1,16]
#   xs[h,m,r] = x^T[h, 4r+m] * recip  (DVE), DMA-scatter into y^T tile
#   out[r, o] = sum_f y^T[f,r] x W_out^T[f,o] + b_out  (fp32r matmuls)
import numpy as np

B, S, F, H = 2, 1024, 1024, 16
NG = 16          # groups per core
P = 128
NCORES = 8


def _fp32r(x):
    """Round fp32 -> fp32r (11-bit mantissa, round-half-up) like the HW expects."""
    b = np.ascontiguousarray(x.astype(np.float32)).view(np.uint32)
    r = ((b.astype(np.uint64) + 0x800) & 0xFFFFF000).astype(np.uint32)
    return r.view(np.float32)


def _core_groups(c):
    b2, qq = c // 4, c % 4
    b = qq % 2
    jmod = 2 * b2 + qq // 2
    js = [4 * h2 + jmod for h2 in range(NG)]
    return b2, qq, b, js


def _build(causal: bool, n_iter: int = 1):
    import concourse.bass as bass
    import concourse.mybir as mybir
    from concourse import bacc, tile

    F32 = mybir.dt.float32
    F32R = mybir.dt.float16   # attention/linear operand dtype (full-rate PE)
    AF = mybir.ActivationFunctionType
    ADD = mybir.AluOpType.add
    MUL = mybir.AluOpType.mult

    nc = bacc.Bacc("TRN2", target_bir_lowering=False, debug=False)
    qt = nc.declare_dram_parameter("qt", [NG * H, S], F32R, isOutput=False)
    kt = nc.declare_dram_parameter("kt", [NG * H, S], F32R, isOutput=False)
    va = nc.declare_dram_parameter("va", [S, NG * 33], F32R, isOutput=False)
    wt = nc.declare_dram_parameter("wt", [F, F], F32R, isOutput=False)
    msk = nc.declare_dram_parameter("msk", [P, 256], F32, isOutput=False)
    bb = nc.declare_dram_parameter("bb", [P, F], F32, isOutput=False)
    out = nc.declare_dram_parameter("o", [256, F], F32, isOutput=True)

    NT = S // P           # 8 s2 tiles
    NC_ = S // 512        # 2 s1 chunks

    import contextlib
    with tile.TileContext(nc) as tc:
        loop_ctx = tc.For_i(0, n_iter, 1, hint_engines=(
            mybir.EngineType.PE, mybir.EngineType.DVE, mybir.EngineType.Activation,
            mybir.EngineType.SP, mybir.EngineType.Pool,
        )) if n_iter > 1 else contextlib.nullcontext()
        with loop_ctx, \
             tc.tile_pool(name="cst", bufs=1) as cst, \
             tc.tile_pool(name="qk", bufs=4) as qkp, \
             tc.tile_pool(name="expp", bufs=3) as expp, \
             tc.tile_pool(name="work", bufs=3) as wkp, \
             tc.tile_pool(name="yt", bufs=1) as ytp, \
             tc.tile_pool(name="stps", bufs=5, space="PSUM") as stps, \
             tc.tile_pool(name="xtps", bufs=3, space="PSUM") as xtps:

            va_sb = cst.tile([P, NT, NG * 33], F32R)
            wt_sb = cst.tile([P, F // P, F], F32R)
            msk_sb = cst.tile([P, 256], F32)
            bb_sb = cst.tile([P, F], F32)
            nc.sync.dma_start(va_sb[:], va.rearrange("(t p) m -> p t m", p=P))
            nc.sync.dma_start(wt_sb[:], wt.rearrange("(t p) m -> p t m", p=P))
            nc.sync.dma_start(msk_sb[:], msk[:])
            nc.sync.dma_start(bb_sb[:], bb[:])

            yt_sb = ytp.tile([P, F // P, 256], F32R)

            for g in range(NG):
                qt_g = qkp.tile([H, S], F32R, tag="qt")
                kt_g = qkp.tile([H, S], F32R, tag="kt")
                nc.sync.dma_start(qt_g[:], qt[g * H:(g + 1) * H, :])
                nc.sync.dma_start(kt_g[:], kt[g * H:(g + 1) * H, :])
                for c in range(NC_):
                    ntile = 4 * c + 4 if causal else NT
                    expt = expp.tile([P, NT, 512], F32R, tag="expt")
                    xt = xtps.tile([33, 512], F32, tag="xt")
                    for t in range(ntile):
                        st = stps.tile([P, 512], F32, tag="st")
                        d = t - 4 * c
                        a1 = 128 * d if (causal and d >= 0) else 0
                        nc.tensor.matmul(
                            st[:, a1:], kt_g[:, t * P:(t + 1) * P],
                            qt_g[:, 512 * c + a1: 512 * (c + 1)],
                            start=True, stop=True)
                        if causal and d >= 0:
                            nc.vector.tensor_tensor(
                                out=st[:, a1:a1 + P], in0=st[:, a1:a1 + P],
                                in1=msk_sb[:, 128:], op=ADD)
                        nc.scalar.activation(expt[:, t, a1:], st[:, a1:], AF.Exp)
                        nc.tensor.matmul(
                            xt[:, a1:], va_sb[:, t, g * 33:(g + 1) * 33],
                            expt[:, t, a1:],
                            start=(t == 0), stop=(t == ntile - 1))
                    recip = wkp.tile([1, 512], F32R, tag="recip")
                    with nc.allow_low_precision(reason="fp16 softmax recip"):
                        nc.vector.reciprocal(recip[:], xt[32:33, :])
                    recipb = wkp.tile([16, 512], F32R, tag="recipb")
                    nc.gpsimd.partition_broadcast(recipb[:], recip[:])
                    xs = wkp.tile([16, 4, 128], F32R, tag="xs")
                    for m in range(4):
                        nc.vector.tensor_tensor(
                            out=xs[:, m, :], in0=xt[0:16, m:512:4],
                            in1=recipb[:, m:512:4], op=MUL)
                    po = 64 * (g % 2)
                    for m in range(4):
                        nc.sync.dma_start(
                            out=yt_sb[po + 16 * m: po + 16 * (m + 1), g // 2,
                                      128 * c:128 * (c + 1)],
                            in_=xs[:, m, :])

            # output linear: out[r, o] = sum_f yT[f, r] * wt[f, o] + b[o]
            for r2 in range(2):
                for oc in range(2):
                    ps_t = stps.tile([P, 512], F32, tag="st")
                    ps = ps_t[:]
                    for ft in range(F // P):
                        nc.tensor.matmul(
                            ps, yt_sb[:, ft, r2 * P:(r2 + 1) * P],
                            wt_sb[:, ft, oc * 512:(oc + 1) * 512],
                            start=(ft == 0), stop=(ft == F // P - 1))
                    ot = wkp.tile([P, 512], F32, tag="ot")
                    nc.vector.tensor_tensor(
                        out=ot[:], in0=ps, in1=bb_sb[:, oc * 512:(oc + 1) * 512],
                        op=ADD)
                    nc.sync.dma_start(
                        out=out[r2 * P:(r2 + 1) * P, oc * 512:(oc + 1) * 512],
                        in_=ot[:])
    nc.compile()
    return nc


_NC_CACHE = {}


def _get_nc(causal: bool, n_iter: int = 1):
    key = (causal, n_iter)
    if key not in _NC_CACHE:
        _NC_CACHE[key] = _build(causal, n_iter)
    return _NC_CACHE[key]


def _shard_inputs(q, k, v, W_out, b_out):
    """Build the 8 per-core input maps (all fp32r pre-rounded where needed)."""
    wt = np.ascontiguousarray(W_out.T).astype(np.float16)
    mskv = np.full((P, 256), -1e9, np.float32)
    xi, yi = np.mgrid[0:P, 0:P]
    mskv[:, 128:] = np.where(yi >= xi, 0.0, -1e9).astype(np.float32)
    bbv = np.broadcast_to(b_out.astype(np.float32), (P, F)).copy()

    in_maps = []
    for c in range(NCORES):
        _, _, b, js = _core_groups(c)
        cols = np.concatenate([j * H + np.arange(H) for j in js])
        qc = (0.25 * q[b][:, cols].T).astype(np.float16)     # [256, S]
        kc = np.ascontiguousarray(k[b][:, cols].T).astype(np.float16)
        vav = np.zeros((S, NG, 33), np.float32)
        vav[:, :, :16] = v[b][:, cols].reshape(S, NG, H)
        vav[:, :, 32] = 1.0
        in_maps.append({
            "qt": np.ascontiguousarray(qc),
            "kt": kc,
            "va": vav.reshape(S, NG * 33).astype(np.float16),
            "wt": wt,
            "msk": mskv,
            "bb": bbv,
        })
    return in_maps


def _unshard(outs):
    full = np.empty((B, S, F), np.float32)
    for c in range(NCORES):
        b2, qq, _, _ = _core_groups(c)
        full[b2, 256 * qq:256 * (qq + 1), :] = outs[c]
    return full


def _numpy_core(in_map, causal=True):
    """Numpy emulation of the device program (for host-logic validation)."""
    qt = in_map["qt"].astype(np.float32); kt = in_map["kt"].astype(np.float32)
    va = in_map["va"].reshape(S, NG, 33).astype(np.float32)
    wtm = in_map["wt"].astype(np.float32); bbv = in_map["bb"]
    ytv = np.zeros((F, 256), np.float32)
    for g in range(NG):
        sc = kt[g * H:(g + 1) * H].T @ qt[g * H:(g + 1) * H]   # [s2, s1]
        if causal:
            s2i, s1i = np.mgrid[0:S, 0:S]
            sc = np.where(s1i >= s2i, sc, -1e9)
        e = np.exp(sc).astype(np.float16).astype(np.float32)
        if causal:
            e = np.where(s1i >= s2i, e, 0.0).astype(np.float32)
        xt = va[:, g, :].T @ e                                  # [33, s1]
        recip = (1.0 / xt[32]).astype(np.float16).astype(np.float32)
        xs = (xt[0:16] * recip[None, :]).astype(np.float16).astype(np.float32)                  # [h, s1]
        po = 64 * (g % 2)
        for m in range(4):
            for cc in range(2):
                ytv[128 * (g // 2) + po + 16 * m: 128 * (g // 2) + po + 16 * (m + 1),
                    128 * cc:128 * (cc + 1)] = xs[:, 512 * cc + m:512 * (cc + 1):4]
    o = ytv.T @ wtm + bbv[0][None, :]
    return o.astype(np.float32)


def kernel(q, k, v, W_out, b_out, apply_mask, _mock=False):
    q = np.asarray(q, np.float32)
    k = np.asarray(k, np.float32)
    v = np.asarray(v, np.float32)
    W_out = np.asarray(W_out, np.float32)
    b_out = np.asarray(b_out, np.float32)
    causal = bool(int(np.asarray(apply_mask)))
    in_maps = _shard_inputs(q, k, v, W_out, b_out)
    if _mock:
        outs = [_numpy_core(m, causal) for m in in_maps]
        return _unshard(outs)
    from concourse.bass_utils import run_bass_kernel_spmd
    nc = _get_nc(causal)
    res = run_bass_kernel_spmd(nc, in_maps, core_ids=list(range(NCORES)))
    return _unshard([r["o"] for r in res.results])



# revision 56
# speedup vs baseline: 2.9753x; 2.9753x over previous
# Trainium2 Bass kernel for nn_MultiHeadAttentionPure (B=2, S=1024, F=1024, H=16).
#
# The reference splits q/k/v into 64 feature-chunks of 16 ("groups"), runs
# causal attention independently per (group, batch) pair -- 128 independent
# [1024,16] attention problems -- then applies a (buggy-but-faithful) torch
# reshape that scrambles (group, batch, seq) into the [B,S,F] tensor fed to
# the output linear layer.
#
# Sharding: core c = b2*4 + q (q = s2 block of 256) needs exactly the 16
# groups {j : j%4 == 2*b2 + q//2} at input batch b = q%2 -- a perfect
# partition of the 128 (group, batch) pairs across 8 cores with zero
# cross-core traffic.  Each core computes its 16 attention groups, assembles
# its y^T tile ([1024 features, 256 rows]) on-chip, and runs the output
# linear for its 256 output rows.  Host slices inputs / concats outputs.
#
# On-device layout (per core, per group g), causal path:
#   - the 8 causal-diagonal score slabs (s1-widths 512/384/256/128 per
#     chunk) are matmul'd into ONE packed PSUM region [128, 2560] (5 banks)
#     so a single Exp activation covers all of them (ACT per-instr overhead
#     dominates otherwise); the 4 full slabs of chunk c=1 use per-tile
#     [128,512] PSUM tiles.
#   - causal triangles: gpsimd (Pool) adds a -1e9 triangle onto the 8
#     diagonal 128x128 sub-blocks of the packed region before Exp.
#   - x^T accumulation: va-aug stationaries [128,34] ([va(16)+ones | 0] for
#     chunk 0, [0 | va+ones] for chunk 1) accumulate both chunks into one
#     PSUM bank xt [34, 512]; row 16/33 = softmax denominator.
#   - normalize: DVE reciprocal of the denom row, Pool partition-broadcast
#     to 16 partitions, DVE multiplies (m-deinterleaved into xs [16,4,128]).
#   - scatter: ONE dma per (g, chunk): xs [16,4,128] -> yt partitions
#     64*(g%2)+4h+m (h-major order matches the linearized source).  W_out is
#     permuted on the host to match this feature order, so the output linear
#     is a plain fp16 matmul sweep over yt x wt + bias.
import numpy as np

B, S, F, H = 2, 1024, 1024, 16
NG = 16          # groups per core
P = 128
NCORES = 8
DP = 2560        # packed diagonal-slab region columns (5 PSUM banks)

# The 12 causal score slabs of a group (s1-chunks of 512, s2-tiles of 128)
# are packed into FIVE [128, <=1024] PSUM tiles ("waves"), each covered by a
# single Exp.  Entries: (chunk c, s2-tile t, a1 = xt column offset, width,
# packed column offset within the wave, tri).  Slab pieces never cross a
# PSUM bank (512-col) boundary.  tri=True slabs start with a 128x128 causal
# triangle at their packed offset.
WAVES = [
    [(1, 0, 0, 512, 0, False), (1, 1, 0, 512, 512, False)],
    [(1, 2, 0, 512, 0, False), (1, 3, 0, 512, 512, False)],
    [(0, 0, 0, 512, 0, True), (1, 4, 0, 512, 512, True)],
    [(0, 1, 128, 384, 0, True), (0, 3, 384, 128, 384, True),
     (0, 2, 256, 256, 512, True), (1, 6, 256, 256, 768, True)],
    [(1, 5, 128, 384, 0, True), (1, 7, 384, 128, 384, True)],
]


def _core_groups(c):
    b2, qq = c // 4, c % 4
    b = qq % 2
    jmod = 2 * b2 + qq // 2
    js = [4 * h2 + jmod for h2 in range(NG)]
    return b2, qq, b, js


def _perm_feature(p, blk):
    """Logical yt feature stored at partition p (0..127), block blk (0..7).
    p = 64*g0 + 4*h + m  ->  f = 128*blk + 64*g0 + 16*m + h."""
    g0, m, h = p // 64, p % 4, (p % 64) // 4
    return 128 * blk + 64 * g0 + 16 * m + h


def _build(causal: bool, n_iter: int = 1):
    import concourse.bass as bass
    import concourse.mybir as mybir
    from concourse import bacc, tile

    F32 = mybir.dt.float32
    F16 = mybir.dt.float16
    AF = mybir.ActivationFunctionType
    ADD = mybir.AluOpType.add
    MUL = mybir.AluOpType.mult

    nc = bacc.Bacc("TRN2", target_bir_lowering=False, debug=False)
    qkt = nc.declare_dram_parameter("qkt", [P, 16 * S], F16, isOutput=False)
    va = nc.declare_dram_parameter("va", [P, 8 * 528], F16, isOutput=False)
    wt = nc.declare_dram_parameter("wt", [P, (F // P) * F], F16, isOutput=False)
    # trid[:, 0:128] = -1e4 * [k < p] (strict lower triangle, stationary),
    # trid[:, 128:256] = identity.  tri-mask = trid[:, :128].T @ trid[:, 128:]
    trid = nc.declare_dram_parameter("trid", [P, 2 * P], F16, isOutput=False)
    bb = nc.declare_dram_parameter("bb", [P, F], F32, isOutput=False)
    out = nc.declare_dram_parameter("o", [256, F], F32, isOutput=True)

    NT = S // P           # 8 s2 tiles
    NC_ = 2               # s1 chunks of 512

    import contextlib
    with tile.TileContext(nc) as tc:
        loop_ctx = tc.For_i(0, n_iter, 1, hint_engines=(
            mybir.EngineType.PE, mybir.EngineType.DVE, mybir.EngineType.Activation,
            mybir.EngineType.SP, mybir.EngineType.Pool,
        )) if n_iter > 1 else contextlib.nullcontext()
        with loop_ctx, \
             tc.tile_pool(name="cst", bufs=1) as cst, \
             tc.tile_pool(name="exw", bufs=4) as exw, \
             tc.tile_pool(name="wk", bufs=3) as wkp, \
             tc.tile_pool(name="yt", bufs=1) as ytp, \
             tc.tile_pool(name="pk", bufs=3, space="PSUM") as pkp, \
             tc.tile_pool(name="xps", bufs=2, space="PSUM") as xps:

            qkt_sb = cst.tile([P, 16, S], F16)
            va_sb = cst.tile([P, NT, 528], F16)
            wt_sb = cst.tile([P, F // P, F], F16)
            trid_sb = cst.tile([P, 2 * P], F16)
            bb_sb = cst.tile([P, F], F32)
            nc.sync.dma_start(qkt_sb[:], qkt[:])
            nc.sync.dma_start(va_sb[:], va[:])
            nc.sync.dma_start(wt_sb[:], wt[:])
            nc.sync.dma_start(trid_sb[:], trid[:])
            nc.sync.dma_start(bb_sb[:], bb[:])

            yt_sb = ytp.tile([P, F // P, 256], F16)

            if causal:
                waves = WAVES
            else:
                # non-causal: 16 full slabs, 512-col pieces, waves of 2
                slabs = [(c, t, 0, 512, 512 * (i % 2), False)
                         for i, (c, t) in enumerate(
                             (c, t) for c in range(NC_) for t in range(NT))]
                waves = [slabs[i:i + 2] for i in range(0, 16, 2)]
            NW = len(waves)
            wave_list = [(g, wi) for g in range(NG) for wi in range(NW)]
            n_mm = sum(len(wv) for wv in waves)

            def q_l(g, sl):
                return qkt_sb[64 * (g % 2):64 * (g % 2) + 32, g // 2, sl]

            def k_l(g, sl):
                return qkt_sb[64 * (g % 2):64 * (g % 2) + 32, 8 + g // 2, sl]

            def emit_scores(g, wi):
                """Score matmuls (with matmul'd causal tri-mask pre-bias) +
                one exp for wave wi of g."""
                wv = waves[wi]
                wlen = max(off + w for _, _, _, w, off, _ in wv)
                pkt = pkp.tile([P, 1024], F32, tag="pk", name=f"pk_{g}_{wi}")
                for c, t, a1, w, off, tri in wv:
                    if tri:
                        # causal triangle as a matmul: PSUM <- -1e4*[j<p],
                        # then the score matmul accumulates on top.
                        nc.tensor.matmul(
                            pkt[:, off:off + P], trid_sb[:, 0:P],
                            trid_sb[:, P:2 * P], start=True, stop=False)
                        nc.tensor.matmul(
                            pkt[:, off:off + P],
                            k_l(g, slice(t * P, (t + 1) * P)),
                            q_l(g, slice(512 * c + a1, 512 * c + a1 + P)),
                            start=False, stop=True)
                        if w > P:
                            nc.tensor.matmul(
                                pkt[:, off + P:off + w],
                                k_l(g, slice(t * P, (t + 1) * P)),
                                q_l(g, slice(512 * c + a1 + P, 512 * (c + 1))),
                                start=True, stop=True)
                    else:
                        nc.tensor.matmul(
                            pkt[:, off:off + w],
                            k_l(g, slice(t * P, (t + 1) * P)),
                            q_l(g, slice(512 * c + a1, 512 * (c + 1))),
                            start=True, stop=True)
                ew = exw.tile([P, 1024], F16, tag="ew", name=f"ew_{g}_{wi}")
                nc.scalar.activation(ew[:, 0:wlen], pkt[:, 0:wlen], AF.Exp)
                return ew

            xt_of = {}
            # per-chunk xmm counts for start/stop flags
            n_mm_c = [sum(1 for wv in waves for s in wv if s[0] == c)
                      for c in range(NC_)]

            def emit_xmms(g, wi, ew):
                """x^T accumulation for wave wi of group g (lag pipelined).
                The 33-col stationary is [features(16) | zeros(16) | ones]:
                chunk 0 lands at xt partitions [0:33] (denominator row 32),
                chunk 1 at [64:97] (denominator row 96) -- feature reads at
                base 0/64 and denom reads at base 32/96 keep every engine AP
                32-partition aligned."""
                if wi == 0:
                    xt_of[g] = [xps.tile([P, 512], F32, tag="xt",
                                         name=f"xt_{g}"), [0] * NC_]
                xt, cnt = xt_of[g]
                for c, t, a1, w, off, tri in waves[wi]:
                    fo = 64 * c
                    nc.tensor.matmul(
                        xt[fo:fo + 33, a1:a1 + w] if causal
                        else xt[fo:fo + 33, 0:512],
                        va_sb[:, t, 33 * g:33 * g + 33], ew[:, off:off + w],
                        start=(cnt[c] == 0), stop=(cnt[c] == n_mm_c[c] - 1))
                    cnt[c] += 1

            def emit_normalize(g):
                """softmax-normalize + scatter both chunks of group g."""
                xt, _ = xt_of.pop(g)
                for c in range(NC_):
                    fo = 64 * c     # feature/denom partition offset in xt
                    recip = wkp.tile([1, 512], F16, tag="recip",
                                     name=f"rc_{g}_{c}")
                    with nc.allow_low_precision(reason="fp16 softmax recip"):
                        nc.vector.reciprocal(recip[:], xt[fo + 32:fo + 33, :])
                    recipb = wkp.tile([16, 512], F16, tag="recipb",
                                      name=f"rb_{g}_{c}")
                    nc.gpsimd.partition_broadcast(recipb[:], recip[:])
                    xs = wkp.tile([16, 4, 128], F16, tag="xs",
                                  name=f"xs_{g}_{c}")
                    for m in range(4):
                        nc.vector.tensor_tensor(
                            out=xs[:, m, :], in0=xt[fo:fo + 16, m:512:4],
                            in1=recipb[:, m:512:4], op=MUL)
                    g0 = g % 2
                    nc.sync.dma_start(
                        out=yt_sb[64 * g0:64 * g0 + 64, g // 2,
                                  128 * c:128 * (c + 1)],
                        in_=xs[:])

            # software pipeline: scores(w) || xmms(w-LAG), normalize at
            # group boundaries
            LAG = 3
            ew_of = {}
            for w, (g, wi) in enumerate(wave_list):
                ew_of[(g, wi)] = emit_scores(g, wi)
                if w >= LAG:
                    gp, wip = wave_list[w - LAG]
                    emit_xmms(gp, wip, ew_of.pop((gp, wip)))
                    if wip == NW - 1:
                        emit_normalize(gp)
            for w in range(len(wave_list) - LAG, len(wave_list)):
                gp, wip = wave_list[w]
                emit_xmms(gp, wip, ew_of.pop((gp, wip)))
                if wip == NW - 1:
                    emit_normalize(gp)

            # output linear: out[r, o] = sum_f yT[f, r] * wt[f, o] + b[o]
            for r2 in range(2):
                for oc in range(2):
                    ps_t = xps.tile([P, 512], F32, tag="xt")
                    ps = ps_t[:]
                    for ft_i in range(F // P):
                        nc.tensor.matmul(
                            ps, yt_sb[:, ft_i, r2 * P:(r2 + 1) * P],
                            wt_sb[:, ft_i, oc * 512:(oc + 1) * 512],
                            start=(ft_i == 0), stop=(ft_i == F // P - 1))
                    ot = wkp.tile([P, 512], F32, tag="ot")
                    nc.vector.tensor_tensor(
                        out=ot[:], in0=ps, in1=bb_sb[:, oc * 512:(oc + 1) * 512],
                        op=ADD)
                    nc.sync.dma_start(
                        out=out[r2 * P:(r2 + 1) * P, oc * 512:(oc + 1) * 512],
                        in_=ot[:])
    nc.compile()
    return nc


_NC_CACHE = {}


def _get_nc(causal: bool, n_iter: int = 1):
    key = (causal, n_iter)
    if key not in _NC_CACHE:
        _NC_CACHE[key] = _build(causal, n_iter)
    return _NC_CACHE[key]


def _shard_inputs(q, k, v, W_out, b_out):
    """Build the 8 per-core input maps."""
    wtT = np.ascontiguousarray(W_out.T).astype(np.float32)   # [f_in, o]
    wtp = np.empty((P, F // P, F), np.float16)
    for blk in range(F // P):
        for p in range(P):
            wtp[p, blk, :] = wtT[_perm_feature(p, blk), :]
    wtp = wtp.reshape(P, (F // P) * F)

    ki, pi = np.mgrid[0:P, 0:P]
    tridv = np.zeros((P, 2 * P), np.float16)
    tridv[:, 0:P] = np.where(ki < pi, -1e4, 0.0)      # strict lower tri
    tridv[:, P:2 * P] = np.where(ki == pi, 1.0, 0.0)  # identity
    bbv = np.broadcast_to(b_out.astype(np.float32), (P, F)).copy()

    in_maps = []
    for core in range(NCORES):
        _, _, b, js = _core_groups(core)
        cols = np.stack([j * H + np.arange(H) for j in js])    # [16, 16]
        qkt = np.zeros((P, 16, S), np.float16)
        vav = np.zeros((P, NT_ := S // P, 528), np.float16)
        for g in range(NG):
            po, pb = 64 * (g % 2), g // 2
            qkt[po:po + 16, pb, :] = (0.25 * q[b][:, cols[g]].T).astype(np.float16)
            qkt[po:po + 16, 8 + pb, :] = k[b][:, cols[g]].T.astype(np.float16)
            vg = v[b][:, cols[g]].astype(np.float16)           # [S, 16]
            vg = vg.reshape(NT_, P, H).transpose(1, 0, 2)      # [P, NT, 16]
            vav[:, :, 33 * g:33 * g + 16] = vg
            vav[:, :, 33 * g + 32] = 1.0
        in_maps.append({
            "qkt": qkt.reshape(P, 16 * S),
            "va": vav.reshape(P, (S // P) * 528),
            "wt": wtp,
            "trid": tridv,
            "bb": bbv,
        })
    return in_maps


def _unshard(outs):
    full = np.empty((B, S, F), np.float32)
    for c in range(NCORES):
        b2, qq, _, _ = _core_groups(c)
        full[b2, 256 * qq:256 * (qq + 1), :] = outs[c]
    return full


def _numpy_core(in_map, causal=True):
    """Numpy emulation of the device program (host-logic validation)."""
    qkt = in_map["qkt"].reshape(P, 16, S).astype(np.float32)
    va = in_map["va"].reshape(P, S // P, 528).astype(np.float32)
    wtm = in_map["wt"].reshape(P, F // P, F).astype(np.float32)
    bbv = in_map["bb"]
    ytv = np.zeros((P, F // P, 256), np.float32)
    for g in range(NG):
        po, pb = 64 * (g % 2), g // 2
        qt = qkt[po:po + 16, pb, :]       # [16, S], pre-scaled
        kt = qkt[po:po + 16, 8 + pb, :]
        sc = kt.T @ qt                    # [s2, s1]
        if causal:
            s2i, s1i = np.mgrid[0:S, 0:S]
            sc = np.where(s1i >= s2i, sc, -1e9)
        e = np.exp(sc).astype(np.float16).astype(np.float32)
        if causal:
            e = np.where(s1i >= s2i, e, 0.0).astype(np.float32)
        # x accumulation with augmented va (feature rows + ones row at 32)
        vg = np.zeros((S, 33), np.float32)
        for t in range(S // P):
            vg[t * P:(t + 1) * P, :] = va[:, t, 33 * g:33 * g + 33]
        xt = vg.T @ e                     # [33, s1]
        recip = (1.0 / xt[32]).astype(np.float16).astype(np.float32)
        xs = (xt[0:16] * recip[None, :]).astype(np.float16).astype(np.float32)
        g0 = g % 2
        for c in range(2):
            for h in range(16):
                for m in range(4):
                    p = 64 * g0 + 4 * h + m
                    ytv[p, g // 2, 128 * c:128 * (c + 1)] = \
                        xs[h, 512 * c + m:512 * (c + 1):4]
    yt2 = ytv.reshape(P, (F // P) * 256)
    o = np.zeros((256, F), np.float32)
    for r2 in range(2):
        for oc in range(2):
            acc = np.zeros((P, 512), np.float32)
            for ft_i in range(F // P):
                acc += ytv[:, ft_i, r2 * P:(r2 + 1) * P].T @ \
                    wtm[:, ft_i, oc * 512:(oc + 1) * 512]
            o[r2 * P:(r2 + 1) * P, oc * 512:(oc + 1) * 512] = \
                acc + bbv[0][None, oc * 512:(oc + 1) * 512]
    return o.astype(np.float32)


def kernel(q, k, v, W_out, b_out, apply_mask, _mock=False):
    q = np.asarray(q, np.float32)
    k = np.asarray(k, np.float32)
    v = np.asarray(v, np.float32)
    W_out = np.asarray(W_out, np.float32)
    b_out = np.asarray(b_out, np.float32)
    causal = bool(int(np.asarray(apply_mask)))
    in_maps = _shard_inputs(q, k, v, W_out, b_out)
    if _mock:
        outs = [_numpy_core(m, causal) for m in in_maps]
        return _unshard(outs)
    from concourse.bass_utils import run_bass_kernel_spmd
    nc = _get_nc(causal)
    res = run_bass_kernel_spmd(nc, in_maps, core_ids=list(range(NCORES)))
    return _unshard([r["o"] for r in res.results])
